# revision 29
# baseline (speedup 1.0000x reference)
"""Trainium2 Bass kernel for masked attention softmax (ragged sequences).

Reference computation (per batch b):
    qp[k]   = sum_q query[b,0,q] * w[k,q]
    att[s]  = sum_k qp[k] * keys[b,s,k]
    score   = where(s < seq_len[b], att, NEG_INF)
    out[b]  = softmax(score)            # over s axis

v4: PE+DVE split compute (v3) with a rebuilt DMA system.

  - Host sorts batches by seq_len descending; core c's slot s holds
    batch order[8*s + c], so slot extents (hardcoded at build time
    from slot 0's core-0 batch) bound every core's batch.
  - PE path (slots 0..255): per batch one self-loading matmul with
    the batch's transposed keys [k=128, E] as the stationary operand
    and its projected query qpT[:,i] as a 1-column moving operand ->
    one PSUM column.  128 batches fill a [s, b]-transposed PSUM tile;
    ACT exps whole tiles; host un-transposes during the unshard.
    Measured: 0.833ns/weight-col + ~37ns/matmul.
  - DVE path (v1 fp16 chunked multiply + halving-tree reduce):
    3 partition tiles; tile 0 is the HEAD [0, XHEAD) of slots
    128..255 whose tails [XHEAD, E) run on the PE, tiles 1-2 are
    slots 256..511 in full.  XHEAD picked at prep to equalize
    predicted engine busy (~36us each) under the ~42us DMA roofline.
  - DMA: v3 put the DVE keys on the sync HWDGE ring, which serializes
    transfers (one in flight + ~2us completion receipt each) and
    stretched that stream to ~70us.  v4 issues ALL key chunks on the
    SWDGE (gpsimd) queue -- fire-and-forget descriptor generation, no
    serialization -- explicitly interleaved in compute need-time
    order so both engines are fed continuously at full HBM rate.
    Every issue is dependency-free: the DVE key tiles are fully
    SBUF-resident (3 tiles, 60KB/partition) and chunks DMA into
    disjoint subtile slices, so no pool recycling gates an issue.
    The qp broadcast for the DVE multiply reads the qp tile directly
    (v1-v3 copied qp into each key tile's row 0 on ACT; that ACT->DVE
    dependency is gone).  wq/qw header loads ride the otherwise-empty
    scalar/sync HWDGE rings.  Outputs go on SWDGE behind all inputs.
  - Softmax normalization (and masking) on the host during unshard.
"""

import sys

import numpy as np

sys.path.insert(0, "/opt/trn_rl_repo")

import concourse.bass as bass
import concourse.tile as tile
from concourse import bacc, mybir
from concourse.bass_utils import run_bass_kernel_spmd


def _install_trace_shims():
    """The agent image lacks ``antenv.axon_hooks``, so trace=True silently
    degrades.  Recreate the module and register the ctypes NTFF hook from
    trn_agent_boot; also make artifact upload failure non-fatal."""
    try:
        import types

        import antenv
        from concourse import bass_utils as _bu

        if "antenv.axon_hooks" not in sys.modules:
            mod = types.ModuleType("antenv.axon_hooks")
            mod._hook = None
            mod.set_axon_ntff_profile_hook = lambda h: setattr(mod, "_hook", h)
            mod.get_axon_ntff_profile_hook = lambda: mod._hook
            sys.modules["antenv.axon_hooks"] = mod
            antenv.axon_hooks = mod
            from trn_agent_boot.trn_boot import _ntff_profile_via_ctypes

            mod.set_axon_ntff_profile_hook(
                _ntff_profile_via_ctypes("/opt/axon/libaxon_pjrt.so")
            )

        _orig_upload = _bu.upload_artifacts

        def _safe_upload(tmpdir):
            try:
                return _orig_upload(tmpdir)
            except Exception:
                return "local://" + str(tmpdir)

        _bu.upload_artifacts = _safe_upload
    except Exception:
        pass


_install_trace_shims()

B, S, KD, QD = 4096, 200, 128, 128
NCORES = 8
P = 128
PB = B // NCORES           # batches per core (512)
CH = 38                    # s-positions per DVE keys DMA chunk (9.9KB/part descriptors)
KDA = KD + 2               # zero-padded to 130 (v1: bank-stagger stride)

PE_NSLOTS = 256            # per core; slots 128.. are strip-split at XHEAD
PE_GROUP = 128
PE_NGROUPS = PE_NSLOTS // PE_GROUP
PE_CHUNK_MAX = 4992        # cols per keysT DMA chunk (9.75KB/partition)
PE_CHUNK_FIRST = 1664      # small first chunk so the PE starts early
NTILES_DVE = 3             # head tile (slots 128..255) + slots 256..511

# measured cost constants (v2/v3 traces) for scheduling + balance
_PE_NS_PER_COL = 0.833
_PE_NS_PER_MM = 30.0
_DVE_NS_PER_POS = 163.0

LAST_RESULTS = None
_nc_cache = {}


def _dve_chunks(j, E, last):
    """Chunk schedule for DVE tile j: geometric ramp-up on tile 0 so the
    DVE starts as soon as the first keys land; ramp-DOWN at the end of
    the last tile so the post-DMA compute tail is short."""
    out = []
    c0 = 0
    if j == 0:
        for ch in (8, 16, 26):
            if c0 + ch > E:
                break
            out.append((c0, ch))
            c0 += ch
    tail = []
    rem_end = E
    if last:
        for ch in (8, 16, 26):
            if rem_end - ch <= c0:
                break
            tail.append((rem_end - ch, ch))
            rem_end -= ch
        tail.reverse()
    while c0 < rem_end:
        ch = min(CH, rem_end - c0)
        out.append((c0, ch))
        c0 += ch
    return out + tail


def _pe_widths(pe_exts, xhead):
    """Per-slot weight-column counts: full extent for slots < 128, the
    [xhead, E) strip for slots >= 128."""
    w = []
    for i, e in enumerate(pe_exts):
        w.append(e if i < PE_GROUP else max(0, e - xhead))
    return w


def _pe_chunks(widths):
    """Pack PE slots into DMA chunks of <= PE_CHUNK_MAX cols (slot-
    aligned), tapering the last chunks.  Returns (chunks, off)."""
    n = len(widths)
    total = sum(widths)
    targets = [PE_CHUNK_FIRST, PE_CHUNK_FIRST + 832]
    rem = total - sum(targets)
    while rem > PE_CHUNK_MAX * 1.5:
        targets.append(PE_CHUNK_MAX)
        rem -= PE_CHUNK_MAX
    targets += [int(rem * 0.5), int(rem * 0.3), rem]
    chunks = []
    off = [0] * n
    lo = 0
    cols = 0
    ti = 0
    for i, e in enumerate(widths):
        if cols + e > targets[min(ti, len(targets) - 1)] and cols > 0:
            chunks.append((lo, i, cols))
            ti += 1
            lo, cols = i, 0
        off[i] = cols
        cols += e
    chunks.append((lo, n, cols))
    return chunks, off


def _dma_order(pe_chunks, widths, dve_sched, skip=()):
    """Merge the two key streams by compute need-time (cumulative engine
    busy before each chunk is consumed).  Returns a list of
    ('pk', n) / ('kt', j, ci) in SWDGE issue order.  Units in `skip`
    are carried on the HWDGE rings instead (their clock still
    advances)."""
    units = []
    # kt chunks get a 3us head start so the DVE (which starts later and
    # has no deep chunk buffer ahead of it) never waits at a boundary
    t = -3000.0
    for j in range(NTILES_DVE):
        for ci, (c0, ch) in enumerate(dve_sched[j]):
            if ("kt", j, ci) not in skip:
                units.append((t, 0, ("kt", j, ci)))
            t += ch * _DVE_NS_PER_POS
    t = 0.0
    for n, (lo, hi, cols) in enumerate(pe_chunks):
        if ("pk", n) not in skip:
            units.append((t, 1, ("pk", n)))
        nmm = sum(
            (2 if (i < PE_GROUP and widths[i] > P) else 1)
            for i in range(lo, hi)
            if widths[i] > 0
        )
        t += cols * _PE_NS_PER_COL + nmm * _PE_NS_PER_MM
    units.sort(key=lambda u: (u[0], u[1]))
    return [u[2] for u in units]


def _build(dve_exts, pe_exts, xhead):
    f16 = mybir.dt.float16
    f32 = mybir.dt.float32
    mult = mybir.AluOpType.mult
    add = mybir.AluOpType.add
    nc = bacc.Bacc("TRN2", target_bir_lowering=False, debug=False)

    widths = _pe_widths(pe_exts, xhead)
    pe_chunks, pe_off = _pe_chunks(widths)
    brow0 = max(0, max(pe_exts[:PE_GROUP]) - P)

    # ---- DRAM tensors
    pk_d = [
        nc.dram_tensor(f"pk{n}", [P, cols], f16, kind="ExternalInput")
        for n, (_, _, cols) in enumerate(pe_chunks)
    ]
    # combined header: [ w^T | queryT(PE) | qw(DVE, 3 tiles) ]
    HW = KD + PE_NSLOTS
    hdr_d = nc.dram_tensor(
        "hdr", [QD, HW + NTILES_DVE * (P + KD)], f16, kind="ExternalInput"
    )
    # single combined PE output: [eA0 | eA1 | eB0] column blocks
    pe_ncols = PE_NGROUPS * PE_GROUP + (PE_GROUP if brow0 > 0 else 0)
    pe_e_d = nc.dram_tensor("pe_e", [P, pe_ncols], f32, kind="ExternalOutput")
    keys_c = {}
    dve_sched = {}
    for j in range(NTILES_DVE):
        dve_sched[j] = _dve_chunks(j, dve_exts[j], j == NTILES_DVE - 1)
        for ci, (c0, ch) in enumerate(dve_sched[j]):
            keys_c[(j, ci)] = nc.dram_tensor(
                f"k{j}_{ci}", [P, ch, KDA], f16, kind="ExternalInput"
            )
    e_d = nc.dram_tensor("e", [NTILES_DVE * P, S], f32, kind="ExternalOutput")

    with tile.TileContext(nc) as tc:
        with (
            tc.tile_pool(name="pek", bufs=1) as pek,
            tc.tile_pool(name="keys", bufs=1) as keysp,
            tc.tile_pool(name="prod", bufs=2) as prodp,
            tc.tile_pool(name="tree", bufs=2) as treep,
            tc.tile_pool(name="small", bufs=2) as smallp,
            tc.tile_pool(name="qpp", bufs=NTILES_DVE) as qpp,
            tc.tile_pool(name="pemisc", bufs=1) as pemisc,
            tc.tile_pool(name="psum", bufs=2, space=bass.MemorySpace.PSUM) as psump,
            tc.tile_pool(name="pepsum", bufs=1, space=bass.MemorySpace.PSUM) as pepsum,
        ):
            # ---- header loads.  wq rides the scalar HWDGE ring ALONE:
            # a HWDGE dma_start blocks its issuing engine's FIFO until
            # the transfer completes, and the ACT's next instruction
            # (the qpT copy) waits on wq anyway.  qw rides sync, ahead
            # of the output DMAs.
            ktiles = [
                pek.tile([P, cols], f16, tag=f"pk{n}", name=f"pkt{n}")
                for n, (_, _, cols) in enumerate(pe_chunks)
            ]
            dtiles = [
                keysp.tile(
                    [P, dve_exts[j], KDA], f16, tag=f"kt{j}", name=f"dkt{j}"
                )
                for j in range(NTILES_DVE)
            ]
            # ---- EVERYTHING (headers + all key chunks) on SWDGE in
            # need-time order -- HWDGE dma_starts block their issuing
            # engine and delay the SWDGE stream start; with no HWDGE
            # input traffic the SDMA engines reach the SWDGE queue as
            # soon as the Q7 emits descriptors.  Every issue is
            # dependency-free (distinct PE chunk tiles; fully resident
            # DVE key tiles with chunks in disjoint subtile slices).
            hdr = pemisc.tile(
                [QD, HW + NTILES_DVE * (P + KD)], f16, tag="hdr", name="hdr"
            )
            h1 = HW + (P + KD)
            nc.gpsimd.dma_start(hdr[:, 0:h1], hdr_d[:, 0:h1])
            wq = hdr[:, 0:HW]
            hdr1_emitted = False
            for idx, unit in enumerate(
                _dma_order(pe_chunks, widths, dve_sched)
            ):
                if idx == 3 and not hdr1_emitted:
                    nc.gpsimd.dma_start(hdr[:, h1:], hdr_d[:, h1:])
                    hdr1_emitted = True
                if unit[0] == "pk":
                    n = unit[1]
                    nc.gpsimd.dma_start(ktiles[n][:], pk_d[n][:])
                else:
                    _, j, ci = unit
                    c0, ch = dve_sched[j][ci]
                    nc.gpsimd.dma_start(
                        dtiles[j][:, c0 : c0 + ch, :], keys_c[(j, ci)][:]
                    )
            if not hdr1_emitted:
                nc.gpsimd.dma_start(hdr[:, h1:], hdr_d[:, h1:])

            # ---- qpT for the PE path: qpT[k, i] = sum_q w[k,q] qT[q, i]
            qpT_ps = pepsum.tile([P, PE_NSLOTS], f32, tag="qpT_ps")
            nc.tensor.matmul(
                qpT_ps[:], wq[:, :KD], wq[:, KD:], start=True, stop=True
            )
            qpT = pemisc.tile([P, PE_NSLOTS], f16, tag="qpT")
            nc.scalar.copy(qpT[:], qpT_ps[:])  # f32 -> f16 on ACT

            # ---- DVE-path qp per tile (3D so it can broadcast over s)
            qps = []
            for j in range(NTILES_DVE):
                qb = HW + j * (P + KD)
                qp_ps = psump.tile([P, KD], f32, tag="qp_ps")
                nc.tensor.matmul(
                    qp_ps[:], hdr[:, qb : qb + P], hdr[:, qb + P : qb + P + KD],
                    start=True, stop=True,
                )
                qp = qpp.tile([P, 1, KDA], f16, tag=f"qp{j}", name=f"qp{j}")
                nc.scalar.copy(qp[:, 0, :KD], qp_ps[:])
                qps.append(qp)

            # ---- PE per-slot matmuls (PE queue only)
            psA = [
                pepsum.tile([P, PE_GROUP], f32, tag=f"psA{g}", name=f"psA{g}")
                for g in range(PE_NGROUPS)
            ]
            psB0 = (
                pepsum.tile([brow0, PE_GROUP], f32, tag="psB0", name="psB0")
                if brow0 > 0
                else None
            )
            for n, (lo, hi, cols) in enumerate(pe_chunks):
                kt = ktiles[n]
                for i in range(lo, hi):
                    wd = widths[i]
                    if wd <= 0:
                        continue
                    g = i // PE_GROUP
                    col = i % PE_GROUP
                    o = pe_off[i]
                    if g == 0:
                        ea = min(wd, P)
                        nc.tensor.matmul(
                            psA[0][0:ea, col : col + 1],
                            kt[:, o : o + ea],
                            qpT[:, i : i + 1],
                            start=True, stop=True,
                        )
                        if wd > P:
                            nc.tensor.matmul(
                                psB0[0 : wd - P, col : col + 1],
                                kt[:, o + P : o + wd],
                                qpT[:, i : i + 1],
                                start=True, stop=True,
                            )
                    else:
                        nc.tensor.matmul(
                            psA[1][0:wd, col : col + 1],
                            kt[:, o : o + wd],
                            qpT[:, i : i + 1],
                            start=True, stop=True,
                        )

            # ---- DVE-path main loop
            for j in range(NTILES_DVE):
                E = dve_exts[j]
                qp = qps[j]
                kt = dtiles[j]
                att = smallp.tile([P, E], f32, tag="att", bufs=3)
                e_t = smallp.tile([P, E], f32, tag="e", bufs=3)
                for ci, (c0, ch) in enumerate(dve_sched[j]):
                    prod = prodp.tile([P, CH, KDA], f16, tag="prod")
                    qp_b = qp[:, 0:1, 0:KD].broadcast_to([P, ch, KD])
                    nc.vector.tensor_tensor(
                        prod[:, :ch, 0:KD], kt[:, c0 : c0 + ch, 0:KD], qp_b,
                        op=mult,
                    )
                    r1 = treep.tile([P, CH, 64], f16, tag="r1")
                    nc.vector.tensor_tensor(
                        r1[:, :ch, :], prod[:, :ch, 0:64], prod[:, :ch, 64:128],
                        op=add,
                    )
                    r2 = treep.tile([P, CH, 32], f16, tag="r2")
                    nc.vector.tensor_tensor(
                        r2[:, :ch, :], r1[:, :ch, 0:32], r1[:, :ch, 32:64],
                        op=add,
                    )
                    r3 = treep.tile([P, CH, 16], f16, tag="r3")
                    nc.vector.tensor_tensor(
                        r3[:, :ch, :], r2[:, :ch, 0:16], r2[:, :ch, 16:32],
                        op=add,
                    )
                    r4 = treep.tile([P, CH, 8], f16, tag="r4")
                    nc.vector.tensor_tensor(
                        r4[:, :ch, :], r3[:, :ch, 0:8], r3[:, :ch, 8:16],
                        op=add,
                    )
                    nc.vector.tensor_reduce(
                        att[:, c0 : c0 + ch], r4[:, :ch, :],
                        axis=mybir.AxisListType.X, op=add,
                    )
                    nc.scalar.activation(
                        e_t[:, c0 : c0 + ch],
                        att[:, c0 : c0 + ch],
                        mybir.ActivationFunctionType.Exp,
                        bias=0.0,
                        scale=1.0,
                    )
                # one output DMA per tile on the sync HWDGE ring (the
                # SWDGE Q7 must stay free to generate input descriptors
                # back-to-back -- in v4 the out-issues' exp waits stalled
                # the input stream)
                nc.sync.dma_start(e_d[j * P : (j + 1) * P, 0:E], e_t[:, 0:E])

            # ---- PE-path exp into one combined tile + single out
            e_pe = pemisc.tile([P, pe_ncols], f32, tag="e_pe")
            for g in range(PE_NGROUPS):
                nc.scalar.activation(
                    e_pe[:, g * PE_GROUP : (g + 1) * PE_GROUP], psA[g][:],
                    mybir.ActivationFunctionType.Exp, bias=0.0, scale=1.0,
                )
            if psB0 is not None:
                nc.scalar.activation(
                    e_pe[0:brow0, 2 * PE_GROUP : 3 * PE_GROUP],
                    psB0[0:brow0, :],
                    mybir.ActivationFunctionType.Exp, bias=0.0, scale=1.0,
                )
            # scalar ring: runs in parallel with the last e-outs on sync
            nc.scalar.dma_start(pe_e_d[:], e_pe[:])
    nc.compile()
    return nc


def _balance_xhead(pe_exts_full, e2, e3):
    """Pick XHEAD (even) minimizing max(predicted PE busy, DVE busy)."""
    best = (float("inf"), 64)
    for x in range(32, 128, 2):
        cols = sum(pe_exts_full[:PE_GROUP]) + sum(
            max(0, e - x) for e in pe_exts_full[PE_GROUP:]
        )
        mms = (
            PE_GROUP
            + sum(1 for e in pe_exts_full[:PE_GROUP] if e > P)
            + sum(1 for e in pe_exts_full[PE_GROUP:] if e > x)
        )
        pe = _PE_NS_PER_COL * cols + _PE_NS_PER_MM * mms
        dve = _DVE_NS_PER_POS * (x + e2 + e3)
        m = max(pe, dve)
        if m < best[0]:
            best = (m, x)
    return best[1]


def _prep(query, keys, seq_len, w):
    query = np.ascontiguousarray(np.asarray(query), dtype=np.float32)
    keys = np.asarray(keys)
    w = np.ascontiguousarray(np.asarray(w), dtype=np.float32)
    lens = np.asarray(seq_len).reshape(B).astype(np.int64)

    order = np.argsort(-lens, kind="stable")
    keys16 = keys.astype(np.float16)
    wT16 = np.ascontiguousarray(w.T.astype(np.float16))  # [q, k]
    query16 = query.astype(np.float16)

    def slot_ext(s):
        l = int(lens[order[NCORES * s]])
        return max(2, l + (l & 1))

    pe_exts = tuple(slot_ext(i) for i in range(PE_NSLOTS))
    e2 = min(S, max(1, slot_ext(256)))
    e3 = min(S, max(1, slot_ext(384)))
    xhead = _balance_xhead(pe_exts, e2, e3)
    dve_exts = (xhead, e2, e3)
    widths = _pe_widths(pe_exts, xhead)
    pe_chunks, pe_off = _pe_chunks(widths)
    dve_sched = {
        j: _dve_chunks(j, dve_exts[j], j == NTILES_DVE - 1)
        for j in range(NTILES_DVE)
    }

    in_maps = []
    pe_batches = []
    dve_batches = []
    for c in range(NCORES):
        slots = order[c::NCORES]  # slot s -> batch order[8s + c]
        pb = slots[:PE_NSLOTS]
        db = slots[P:].copy()     # DVE tiles: slots 128..511
        pe_batches.append(pb)
        dve_batches.append(db)

        im = {}
        # combined header: [ w^T | queryT(PE) | qw(DVE) flattened ]
        HW = KD + PE_NSLOTS
        hdr = np.zeros((QD, HW + NTILES_DVE * (P + KD)), dtype=np.float16)
        hdr[:, :KD] = wT16
        hdr[:, KD:HW] = query16[pb, 0, :].T
        qTd = query[db, 0, :].reshape(NTILES_DVE, P, QD).transpose(2, 0, 1)
        for j in range(NTILES_DVE):
            qb = HW + j * (P + KD)
            hdr[:, qb : qb + P] = qTd[:, j, :]
            hdr[:, qb + P : qb + P + KD] = wT16
        im["hdr"] = hdr
        for n, (lo, hi, cols) in enumerate(pe_chunks):
            blk = np.zeros((P, cols), dtype=np.float16)
            for i in range(lo, hi):
                wd = widths[i]
                if wd <= 0:
                    continue
                b = pb[i]
                l = int(lens[b])
                s0 = 0 if i < PE_GROUP else xhead
                if l > s0:
                    o = pe_off[i]
                    blk[:, o : o + (l - s0)] = keys16[b, s0:l, :].T
            im[f"pk{n}"] = blk
        # DVE side
        keys_aug = np.zeros((NTILES_DVE * P, S, KDA), dtype=np.float16)
        keys_aug[:, :, :KD] = keys16[db]
        for j in range(NTILES_DVE):
            for ci, (c0, ch) in enumerate(dve_sched[j]):
                im[f"k{j}_{ci}"] = np.ascontiguousarray(
                    keys_aug[j * P : (j + 1) * P, c0 : c0 + ch, :]
                )
        in_maps.append(im)
    return lens, dve_exts, pe_exts, xhead, pe_batches, dve_batches, in_maps


def kernel(query, keys, seq_len, w):
    global LAST_RESULTS
    (lens, dve_exts, pe_exts, xhead, pe_batches, dve_batches, in_maps) = _prep(
        query, keys, seq_len, w
    )

    key = (dve_exts, pe_exts, xhead)
    nc = _nc_cache.get(key)
    if nc is None:
        nc = _build(dve_exts, pe_exts, xhead)
        _nc_cache[key] = nc

    res = run_bass_kernel_spmd(nc, in_maps, core_ids=list(range(NCORES)))
    LAST_RESULTS = res

    out = np.zeros((B, S), dtype=np.float32)
    for c in range(NCORES):
        r = res.results[c]
        pb = pe_batches[c]
        db = dve_batches[c]
        pe_e = np.asarray(r["pe_e"])
        peA = [
            pe_e[:, g * PE_GROUP : (g + 1) * PE_GROUP]
            for g in range(PE_NGROUPS)
        ]
        peB0 = (
            pe_e[:, 2 * PE_GROUP : 3 * PE_GROUP]
            if pe_e.shape[1] > 2 * PE_GROUP
            else None
        )
        e = np.asarray(r["e"])
        # PE group 0: full rows
        for i in range(PE_GROUP):
            b = pb[i]
            l = int(lens[b])
            if l == 0:
                continue
            if l <= P:
                v = peA[0][:l, i]
            else:
                v = np.concatenate([peA[0][:, i], peB0[: l - P, i]])
            ssum = float(v.sum())
            if ssum == 0.0 or not np.isfinite(ssum):
                ssum = 1.0
            out[b, :l] = v / ssum
        # PE group 1 strips + DVE head tile (tile 0)
        for i in range(PE_GROUP, PE_NSLOTS):
            b = pb[i]
            l = int(lens[b])
            if l == 0:
                continue
            col = i - PE_GROUP
            lh = min(l, xhead)
            head = e[col, :lh]
            if l > xhead:
                v = np.concatenate([head, peA[1][: l - xhead, col]])
            else:
                v = head
            ssum = float(v.sum())
            if ssum == 0.0 or not np.isfinite(ssum):
                ssum = 1.0
            out[b, :l] = v / ssum
        # DVE tiles 1..2 (slots 256..511): plain softmax rows
        for j in range(1, NTILES_DVE):
            E = dve_exts[j]
            rows = db[j * P : (j + 1) * P]
            blk = e[j * P : (j + 1) * P, :E]
            m = (np.arange(E)[None, :] < lens[rows][:, None]).astype(np.float32)
            blk = np.where(m > 0, blk, 0.0)
            ssum = blk.sum(axis=1, keepdims=True)
            ssum[ssum == 0.0] = 1.0
            out[rows, :E] = blk / ssum
    out[lens == 0, :] = np.float32(1.0 / S)
    return out


# revision 30
# speedup vs baseline: 1.0161x; 1.0161x over previous
"""Trainium2 Bass kernel for masked attention softmax (ragged sequences).

Reference computation (per batch b):
    qp[k]   = sum_q query[b,0,q] * w[k,q]
    att[s]  = sum_k qp[k] * keys[b,s,k]
    score   = where(s < seq_len[b], att, NEG_INF)
    out[b]  = softmax(score)            # over s axis

v4: PE+DVE split compute (v3) with a rebuilt DMA system.

  - Host sorts batches by seq_len descending; core c's slot s holds
    batch order[8*s + c], so slot extents (hardcoded at build time
    from slot 0's core-0 batch) bound every core's batch.
  - PE path (slots 0..255): per batch one self-loading matmul with
    the batch's transposed keys [k=128, E] as the stationary operand
    and its projected query qpT[:,i] as a 1-column moving operand ->
    one PSUM column.  128 batches fill a [s, b]-transposed PSUM tile;
    ACT exps whole tiles; host un-transposes during the unshard.
    Measured: 0.833ns/weight-col + ~37ns/matmul.
  - DVE path (v1 fp16 chunked multiply + halving-tree reduce):
    3 partition tiles; tile 0 is the HEAD [0, XHEAD) of slots
    128..255 whose tails [XHEAD, E) run on the PE, tiles 1-2 are
    slots 256..511 in full.  XHEAD picked at prep to equalize
    predicted engine busy (~36us each) under the ~42us DMA roofline.
  - DMA: v3 put the DVE keys on the sync HWDGE ring, which serializes
    transfers (one in flight + ~2us completion receipt each) and
    stretched that stream to ~70us.  v4 issues ALL key chunks on the
    SWDGE (gpsimd) queue -- fire-and-forget descriptor generation, no
    serialization -- explicitly interleaved in compute need-time
    order so both engines are fed continuously at full HBM rate.
    Every issue is dependency-free: the DVE key tiles are fully
    SBUF-resident (3 tiles, 60KB/partition) and chunks DMA into
    disjoint subtile slices, so no pool recycling gates an issue.
    The qp broadcast for the DVE multiply reads the qp tile directly
    (v1-v3 copied qp into each key tile's row 0 on ACT; that ACT->DVE
    dependency is gone).  wq/qw header loads ride the otherwise-empty
    scalar/sync HWDGE rings.  Outputs go on SWDGE behind all inputs.
  - Softmax normalization (and masking) on the host during unshard.
"""

import sys

import numpy as np

sys.path.insert(0, "/opt/trn_rl_repo")

import concourse.bass as bass
import concourse.tile as tile
from concourse import bacc, mybir
from concourse.bass_utils import run_bass_kernel_spmd


def _install_trace_shims():
    """The agent image lacks ``antenv.axon_hooks``, so trace=True silently
    degrades.  Recreate the module and register the ctypes NTFF hook from
    trn_agent_boot; also make artifact upload failure non-fatal."""
    try:
        import types

        import antenv
        from concourse import bass_utils as _bu

        if "antenv.axon_hooks" not in sys.modules:
            mod = types.ModuleType("antenv.axon_hooks")
            mod._hook = None
            mod.set_axon_ntff_profile_hook = lambda h: setattr(mod, "_hook", h)
            mod.get_axon_ntff_profile_hook = lambda: mod._hook
            sys.modules["antenv.axon_hooks"] = mod
            antenv.axon_hooks = mod
            from trn_agent_boot.trn_boot import _ntff_profile_via_ctypes

            mod.set_axon_ntff_profile_hook(
                _ntff_profile_via_ctypes("/opt/axon/libaxon_pjrt.so")
            )

        _orig_upload = _bu.upload_artifacts

        def _safe_upload(tmpdir):
            try:
                return _orig_upload(tmpdir)
            except Exception:
                return "local://" + str(tmpdir)

        _bu.upload_artifacts = _safe_upload
    except Exception:
        pass


_install_trace_shims()

B, S, KD, QD = 4096, 200, 128, 128
NCORES = 8
P = 128
PB = B // NCORES           # batches per core (512)
CH = 38                    # s-positions per DVE keys DMA chunk (9.9KB/part descriptors)
KDA = KD + 2               # zero-padded to 130 (v1: bank-stagger stride)

PE_NSLOTS = 256            # per core; slots 128.. are strip-split at XHEAD
PE_GROUP = 128
PE_NGROUPS = PE_NSLOTS // PE_GROUP
PE_CHUNK_MAX = 4992        # cols per keysT DMA chunk (9.75KB/partition)
PE_CHUNK_FIRST = 1664      # small first chunk so the PE starts early
NTILES_DVE = 3             # head tile (slots 128..255) + slots 256..511

# measured cost constants (v2/v3 traces) for scheduling + balance
_PE_NS_PER_COL = 0.833
_PE_NS_PER_MM = 30.0
_DVE_NS_PER_POS = 163.0

LAST_RESULTS = None
_nc_cache = {}


def _dve_chunks(j, E, last):
    """Chunk schedule for DVE tile j: geometric ramp-up on tile 0 so the
    DVE starts as soon as the first keys land; ramp-DOWN at the end of
    the last tile so the post-DMA compute tail is short."""
    out = []
    c0 = 0
    if j == 0:
        for ch in (8, 16, 26):
            if c0 + ch > E:
                break
            out.append((c0, ch))
            c0 += ch
    tail = []
    rem_end = E
    if last:
        for ch in (8, 16, 26):
            if rem_end - ch <= c0:
                break
            tail.append((rem_end - ch, ch))
            rem_end -= ch
        tail.reverse()
    while c0 < rem_end:
        ch = min(CH, rem_end - c0)
        out.append((c0, ch))
        c0 += ch
    return out + tail


def _pe_widths(pe_exts, xhead):
    """Per-slot weight-column counts: full extent for slots < 128, the
    [xhead, E) strip for slots >= 128."""
    w = []
    for i, e in enumerate(pe_exts):
        w.append(e if i < PE_GROUP else max(0, e - xhead))
    return w


def _pe_chunks(widths):
    """Pack PE slots into DMA chunks of <= PE_CHUNK_MAX cols (slot-
    aligned), tapering the last chunks.  Returns (chunks, off)."""
    n = len(widths)
    total = sum(widths)
    targets = [PE_CHUNK_FIRST, PE_CHUNK_FIRST + 832]
    rem = total - sum(targets)
    while rem > PE_CHUNK_MAX * 1.5:
        targets.append(PE_CHUNK_MAX)
        rem -= PE_CHUNK_MAX
    targets += [int(rem * 0.5), int(rem * 0.3), rem]
    chunks = []
    off = [0] * n
    lo = 0
    cols = 0
    ti = 0
    for i, e in enumerate(widths):
        if cols + e > targets[min(ti, len(targets) - 1)] and cols > 0:
            chunks.append((lo, i, cols))
            ti += 1
            lo, cols = i, 0
        off[i] = cols
        cols += e
    chunks.append((lo, n, cols))
    return chunks, off


def _dma_order(pe_chunks, widths, dve_sched, skip=()):
    """Merge the two key streams by compute need-time (cumulative engine
    busy before each chunk is consumed).  Returns a list of
    ('pk', n) / ('kt', j, ci) in SWDGE issue order.  Units in `skip`
    are carried on the HWDGE rings instead (their clock still
    advances)."""
    units = []
    # kt chunks get a 2us head start so the DVE (which starts later and
    # has no deep chunk buffer ahead of it) never waits at a boundary;
    # the small first pk chunk goes out even earlier so the PE's first
    # matmuls aren't starved behind the kt ramp.
    t = -2000.0
    for j in range(NTILES_DVE):
        for ci, (c0, ch) in enumerate(dve_sched[j]):
            if ("kt", j, ci) not in skip:
                units.append((t, 0, ("kt", j, ci)))
            t += ch * _DVE_NS_PER_POS
    t = -2600.0
    for n, (lo, hi, cols) in enumerate(pe_chunks):
        if ("pk", n) not in skip:
            units.append((t, 1, ("pk", n)))
        nmm = sum(
            (2 if (i < PE_GROUP and widths[i] > P) else 1)
            for i in range(lo, hi)
            if widths[i] > 0
        )
        t += cols * _PE_NS_PER_COL + nmm * _PE_NS_PER_MM
    units.sort(key=lambda u: (u[0], u[1]))
    return [u[2] for u in units]


def _build(dve_exts, pe_exts, xhead):
    f16 = mybir.dt.float16
    f32 = mybir.dt.float32
    mult = mybir.AluOpType.mult
    add = mybir.AluOpType.add
    nc = bacc.Bacc("TRN2", target_bir_lowering=False, debug=False)

    widths = _pe_widths(pe_exts, xhead)
    pe_chunks, pe_off = _pe_chunks(widths)
    brow0 = max(0, max(pe_exts[:PE_GROUP]) - P)

    # ---- DRAM tensors
    pk_d = [
        nc.dram_tensor(f"pk{n}", [P, cols], f16, kind="ExternalInput")
        for n, (_, _, cols) in enumerate(pe_chunks)
    ]
    # combined header: [ w^T | queryT(PE) | qw(DVE, 3 tiles) ]
    HW = KD + PE_NSLOTS
    hdr_d = nc.dram_tensor(
        "hdr", [QD, HW + NTILES_DVE * (P + KD)], f16, kind="ExternalInput"
    )
    # single combined PE output: [eA0 | eA1 | eB0] column blocks
    pe_ncols = PE_NGROUPS * PE_GROUP + (PE_GROUP if brow0 > 0 else 0)
    pe_e_d = nc.dram_tensor("pe_e", [P, pe_ncols], f32, kind="ExternalOutput")
    keys_c = {}
    dve_sched = {}
    for j in range(NTILES_DVE):
        dve_sched[j] = _dve_chunks(j, dve_exts[j], j == NTILES_DVE - 1)
        for ci, (c0, ch) in enumerate(dve_sched[j]):
            keys_c[(j, ci)] = nc.dram_tensor(
                f"k{j}_{ci}", [P, ch, KDA], f16, kind="ExternalInput"
            )
    e_d = nc.dram_tensor("e", [NTILES_DVE * P, S], f32, kind="ExternalOutput")

    with tile.TileContext(nc) as tc:
        with (
            tc.tile_pool(name="pek", bufs=1) as pek,
            tc.tile_pool(name="keys", bufs=1) as keysp,
            tc.tile_pool(name="prod", bufs=2) as prodp,
            tc.tile_pool(name="tree", bufs=2) as treep,
            tc.tile_pool(name="small", bufs=2) as smallp,
            tc.tile_pool(name="qpp", bufs=NTILES_DVE) as qpp,
            tc.tile_pool(name="pemisc", bufs=1) as pemisc,
            tc.tile_pool(name="psum", bufs=2, space=bass.MemorySpace.PSUM) as psump,
            tc.tile_pool(name="pepsum", bufs=1, space=bass.MemorySpace.PSUM) as pepsum,
        ):
            # ---- header loads.  wq rides the scalar HWDGE ring ALONE:
            # a HWDGE dma_start blocks its issuing engine's FIFO until
            # the transfer completes, and the ACT's next instruction
            # (the qpT copy) waits on wq anyway.  qw rides sync, ahead
            # of the output DMAs.
            ktiles = [
                pek.tile([P, cols], f16, tag=f"pk{n}", name=f"pkt{n}")
                for n, (_, _, cols) in enumerate(pe_chunks)
            ]
            dtiles = [
                keysp.tile(
                    [P, dve_exts[j], KDA], f16, tag=f"kt{j}", name=f"dkt{j}"
                )
                for j in range(NTILES_DVE)
            ]
            # ---- EVERYTHING (headers + all key chunks) on SWDGE in
            # need-time order -- HWDGE dma_starts block their issuing
            # engine and delay the SWDGE stream start; with no HWDGE
            # input traffic the SDMA engines reach the SWDGE queue as
            # soon as the Q7 emits descriptors.  Every issue is
            # dependency-free (distinct PE chunk tiles; fully resident
            # DVE key tiles with chunks in disjoint subtile slices).
            hdr = pemisc.tile(
                [QD, HW + NTILES_DVE * (P + KD)], f16, tag="hdr", name="hdr"
            )
            h1 = HW + (P + KD)
            nc.gpsimd.dma_start(hdr[:, 0:h1], hdr_d[:, 0:h1])
            wq = hdr[:, 0:HW]
            hdr1_emitted = False
            for idx, unit in enumerate(
                _dma_order(pe_chunks, widths, dve_sched)
            ):
                if idx == 3 and not hdr1_emitted:
                    nc.gpsimd.dma_start(hdr[:, h1:], hdr_d[:, h1:])
                    hdr1_emitted = True
                if unit[0] == "pk":
                    n = unit[1]
                    nc.gpsimd.dma_start(ktiles[n][:], pk_d[n][:])
                else:
                    _, j, ci = unit
                    c0, ch = dve_sched[j][ci]
                    nc.gpsimd.dma_start(
                        dtiles[j][:, c0 : c0 + ch, :], keys_c[(j, ci)][:]
                    )
            if not hdr1_emitted:
                nc.gpsimd.dma_start(hdr[:, h1:], hdr_d[:, h1:])

            # ---- qpT for the PE path: qpT[k, i] = sum_q w[k,q] qT[q, i]
            qpT_ps = pepsum.tile([P, PE_NSLOTS], f32, tag="qpT_ps")
            nc.tensor.matmul(
                qpT_ps[:], wq[:, :KD], wq[:, KD:], start=True, stop=True
            )
            qpT = pemisc.tile([P, PE_NSLOTS], f16, tag="qpT")
            nc.scalar.copy(qpT[:], qpT_ps[:])  # f32 -> f16 on ACT

            # ---- DVE-path qp per tile (3D so it can broadcast over s)
            qps = []
            for j in range(NTILES_DVE):
                qb = HW + j * (P + KD)
                qp_ps = psump.tile([P, KD], f32, tag="qp_ps")
                nc.tensor.matmul(
                    qp_ps[:], hdr[:, qb : qb + P], hdr[:, qb + P : qb + P + KD],
                    start=True, stop=True,
                )
                qp = qpp.tile([P, 1, KDA], f16, tag=f"qp{j}", name=f"qp{j}")
                nc.scalar.copy(qp[:, 0, :KD], qp_ps[:])
                qps.append(qp)

            # ---- PE per-slot matmuls (PE queue only)
            psA = [
                pepsum.tile([P, PE_GROUP], f32, tag=f"psA{g}", name=f"psA{g}")
                for g in range(PE_NGROUPS)
            ]
            psB0 = (
                pepsum.tile([brow0, PE_GROUP], f32, tag="psB0", name="psB0")
                if brow0 > 0
                else None
            )
            for n, (lo, hi, cols) in enumerate(pe_chunks):
                kt = ktiles[n]
                for i in range(lo, hi):
                    wd = widths[i]
                    if wd <= 0:
                        continue
                    g = i // PE_GROUP
                    col = i % PE_GROUP
                    o = pe_off[i]
                    if g == 0:
                        ea = min(wd, P)
                        nc.tensor.matmul(
                            psA[0][0:ea, col : col + 1],
                            kt[:, o : o + ea],
                            qpT[:, i : i + 1],
                            start=True, stop=True,
                        )
                        if wd > P:
                            nc.tensor.matmul(
                                psB0[0 : wd - P, col : col + 1],
                                kt[:, o + P : o + wd],
                                qpT[:, i : i + 1],
                                start=True, stop=True,
                            )
                    else:
                        nc.tensor.matmul(
                            psA[1][0:wd, col : col + 1],
                            kt[:, o : o + wd],
                            qpT[:, i : i + 1],
                            start=True, stop=True,
                        )

            # ---- DVE-path main loop
            for j in range(NTILES_DVE):
                E = dve_exts[j]
                qp = qps[j]
                kt = dtiles[j]
                att = smallp.tile([P, E], f32, tag="att", bufs=3)
                e_t = smallp.tile([P, E], f32, tag="e", bufs=3)
                for ci, (c0, ch) in enumerate(dve_sched[j]):
                    prod = prodp.tile([P, CH, KDA], f16, tag="prod")
                    qp_b = qp[:, 0:1, 0:KD].broadcast_to([P, ch, KD])
                    nc.vector.tensor_tensor(
                        prod[:, :ch, 0:KD], kt[:, c0 : c0 + ch, 0:KD], qp_b,
                        op=mult,
                    )
                    r1 = treep.tile([P, CH, 64], f16, tag="r1")
                    nc.vector.tensor_tensor(
                        r1[:, :ch, :], prod[:, :ch, 0:64], prod[:, :ch, 64:128],
                        op=add,
                    )
                    r2 = treep.tile([P, CH, 32], f16, tag="r2")
                    nc.vector.tensor_tensor(
                        r2[:, :ch, :], r1[:, :ch, 0:32], r1[:, :ch, 32:64],
                        op=add,
                    )
                    r3 = treep.tile([P, CH, 16], f16, tag="r3")
                    nc.vector.tensor_tensor(
                        r3[:, :ch, :], r2[:, :ch, 0:16], r2[:, :ch, 16:32],
                        op=add,
                    )
                    r4 = treep.tile([P, CH, 8], f16, tag="r4")
                    nc.vector.tensor_tensor(
                        r4[:, :ch, :], r3[:, :ch, 0:8], r3[:, :ch, 8:16],
                        op=add,
                    )
                    nc.vector.tensor_reduce(
                        att[:, c0 : c0 + ch], r4[:, :ch, :],
                        axis=mybir.AxisListType.X, op=add,
                    )
                    nc.scalar.activation(
                        e_t[:, c0 : c0 + ch],
                        att[:, c0 : c0 + ch],
                        mybir.ActivationFunctionType.Exp,
                        bias=0.0,
                        scale=1.0,
                    )
                # one output DMA per tile on the sync HWDGE ring (the
                # SWDGE Q7 must stay free to generate input descriptors
                # back-to-back -- in v4 the out-issues' exp waits stalled
                # the input stream)
                nc.sync.dma_start(e_d[j * P : (j + 1) * P, 0:E], e_t[:, 0:E])

            # ---- PE-path exp into one combined tile + single out
            e_pe = pemisc.tile([P, pe_ncols], f32, tag="e_pe")
            for g in range(PE_NGROUPS):
                nc.scalar.activation(
                    e_pe[:, g * PE_GROUP : (g + 1) * PE_GROUP], psA[g][:],
                    mybir.ActivationFunctionType.Exp, bias=0.0, scale=1.0,
                )
            if psB0 is not None:
                nc.scalar.activation(
                    e_pe[0:brow0, 2 * PE_GROUP : 3 * PE_GROUP],
                    psB0[0:brow0, :],
                    mybir.ActivationFunctionType.Exp, bias=0.0, scale=1.0,
                )
            # scalar ring: runs in parallel with the last e-outs on sync
            nc.scalar.dma_start(pe_e_d[:], e_pe[:])
    nc.compile()
    return nc


def _balance_xhead(pe_exts_full, e2, e3):
    """Pick XHEAD (even) minimizing max(predicted PE busy, DVE busy)."""
    best = (float("inf"), 64)
    for x in range(32, 128, 2):
        cols = sum(pe_exts_full[:PE_GROUP]) + sum(
            max(0, e - x) for e in pe_exts_full[PE_GROUP:]
        )
        mms = (
            PE_GROUP
            + sum(1 for e in pe_exts_full[:PE_GROUP] if e > P)
            + sum(1 for e in pe_exts_full[PE_GROUP:] if e > x)
        )
        pe = _PE_NS_PER_COL * cols + _PE_NS_PER_MM * mms
        dve = _DVE_NS_PER_POS * (x + e2 + e3)
        m = max(pe, dve)
        if m < best[0]:
            best = (m, x)
    return best[1]


def _prep(query, keys, seq_len, w):
    query = np.ascontiguousarray(np.asarray(query), dtype=np.float32)
    keys = np.asarray(keys)
    w = np.ascontiguousarray(np.asarray(w), dtype=np.float32)
    lens = np.asarray(seq_len).reshape(B).astype(np.int64)

    order = np.argsort(-lens, kind="stable")
    keys16 = keys.astype(np.float16)
    wT16 = np.ascontiguousarray(w.T.astype(np.float16))  # [q, k]
    query16 = query.astype(np.float16)

    def slot_ext(s):
        l = int(lens[order[NCORES * s]])
        return max(2, l + (l & 1))

    pe_exts = tuple(slot_ext(i) for i in range(PE_NSLOTS))
    e2 = min(S, max(1, slot_ext(256)))
    e3 = min(S, max(1, slot_ext(384)))
    xhead = _balance_xhead(pe_exts, e2, e3)
    dve_exts = (xhead, e2, e3)
    widths = _pe_widths(pe_exts, xhead)
    pe_chunks, pe_off = _pe_chunks(widths)
    dve_sched = {
        j: _dve_chunks(j, dve_exts[j], j == NTILES_DVE - 1)
        for j in range(NTILES_DVE)
    }

    in_maps = []
    pe_batches = []
    dve_batches = []
    for c in range(NCORES):
        slots = order[c::NCORES]  # slot s -> batch order[8s + c]
        pb = slots[:PE_NSLOTS]
        db = slots[P:].copy()     # DVE tiles: slots 128..511
        pe_batches.append(pb)
        dve_batches.append(db)

        im = {}
        # combined header: [ w^T | queryT(PE) | qw(DVE) flattened ]
        HW = KD + PE_NSLOTS
        hdr = np.zeros((QD, HW + NTILES_DVE * (P + KD)), dtype=np.float16)
        hdr[:, :KD] = wT16
        hdr[:, KD:HW] = query16[pb, 0, :].T
        qTd = query[db, 0, :].reshape(NTILES_DVE, P, QD).transpose(2, 0, 1)
        for j in range(NTILES_DVE):
            qb = HW + j * (P + KD)
            hdr[:, qb : qb + P] = qTd[:, j, :]
            hdr[:, qb + P : qb + P + KD] = wT16
        im["hdr"] = hdr
        for n, (lo, hi, cols) in enumerate(pe_chunks):
            blk = np.zeros((P, cols), dtype=np.float16)
            for i in range(lo, hi):
                wd = widths[i]
                if wd <= 0:
                    continue
                b = pb[i]
                l = int(lens[b])
                s0 = 0 if i < PE_GROUP else xhead
                if l > s0:
                    o = pe_off[i]
                    blk[:, o : o + (l - s0)] = keys16[b, s0:l, :].T
            im[f"pk{n}"] = blk
        # DVE side
        keys_aug = np.zeros((NTILES_DVE * P, S, KDA), dtype=np.float16)
        keys_aug[:, :, :KD] = keys16[db]
        for j in range(NTILES_DVE):
            for ci, (c0, ch) in enumerate(dve_sched[j]):
                im[f"k{j}_{ci}"] = np.ascontiguousarray(
                    keys_aug[j * P : (j + 1) * P, c0 : c0 + ch, :]
                )
        in_maps.append(im)
    return lens, dve_exts, pe_exts, xhead, pe_batches, dve_batches, in_maps


def kernel(query, keys, seq_len, w):
    global LAST_RESULTS
    (lens, dve_exts, pe_exts, xhead, pe_batches, dve_batches, in_maps) = _prep(
        query, keys, seq_len, w
    )

    key = (dve_exts, pe_exts, xhead)
    nc = _nc_cache.get(key)
    if nc is None:
        nc = _build(dve_exts, pe_exts, xhead)
        _nc_cache[key] = nc

    res = run_bass_kernel_spmd(nc, in_maps, core_ids=list(range(NCORES)))
    LAST_RESULTS = res

    out = np.zeros((B, S), dtype=np.float32)
    for c in range(NCORES):
        r = res.results[c]
        pb = pe_batches[c]
        db = dve_batches[c]
        pe_e = np.asarray(r["pe_e"])
        peA = [
            pe_e[:, g * PE_GROUP : (g + 1) * PE_GROUP]
            for g in range(PE_NGROUPS)
        ]
        peB0 = (
            pe_e[:, 2 * PE_GROUP : 3 * PE_GROUP]
            if pe_e.shape[1] > 2 * PE_GROUP
            else None
        )
        e = np.asarray(r["e"])
        # PE group 0: full rows
        for i in range(PE_GROUP):
            b = pb[i]
            l = int(lens[b])
            if l == 0:
                continue
            if l <= P:
                v = peA[0][:l, i]
            else:
                v = np.concatenate([peA[0][:, i], peB0[: l - P, i]])
            ssum = float(v.sum())
            if ssum == 0.0 or not np.isfinite(ssum):
                ssum = 1.0
            out[b, :l] = v / ssum
        # PE group 1 strips + DVE head tile (tile 0)
        for i in range(PE_GROUP, PE_NSLOTS):
            b = pb[i]
            l = int(lens[b])
            if l == 0:
                continue
            col = i - PE_GROUP
            lh = min(l, xhead)
            head = e[col, :lh]
            if l > xhead:
                v = np.concatenate([head, peA[1][: l - xhead, col]])
            else:
                v = head
            ssum = float(v.sum())
            if ssum == 0.0 or not np.isfinite(ssum):
                ssum = 1.0
            out[b, :l] = v / ssum
        # DVE tiles 1..2 (slots 256..511): plain softmax rows
        for j in range(1, NTILES_DVE):
            E = dve_exts[j]
            rows = db[j * P : (j + 1) * P]
            blk = e[j * P : (j + 1) * P, :E]
            m = (np.arange(E)[None, :] < lens[rows][:, None]).astype(np.float32)
            blk = np.where(m > 0, blk, 0.0)
            ssum = blk.sum(axis=1, keepdims=True)
            ssum[ssum == 0.0] = 1.0
            out[rows, :E] = blk / ssum
    out[lens == 0, :] = np.float32(1.0 / S)
    return out


# revision 31
# speedup vs baseline: 1.0657x; 1.0488x over previous
"""Trainium2 Bass kernel for masked attention softmax (ragged sequences).

Reference computation (per batch b):
    qp[k]   = sum_q query[b,0,q] * w[k,q]
    att[s]  = sum_k qp[k] * keys[b,s,k]
    score   = where(s < seq_len[b], att, NEG_INF)
    out[b]  = softmax(score)            # over s axis

Final design (v10): PE+DVE split compute under a need-ordered
single-queue SWDGE input stream.

  - Host sorts batches by seq_len descending; core c's slot s holds
    batch order[8*s + c], so slot extents (hardcoded at build time
    from slot 0's core-0 batch) bound every core's batch.
  - PE path (slots 0..255, the longest half): per batch one
    self-loading matmul with the batch's transposed keys [k=128, E]
    as the stationary operand and its projected query qpT[:,i] as a
    1-column moving operand -> one PSUM column.  128 batches fill a
    [s, b]-transposed PSUM tile; ACT exps whole tiles into one
    combined e_pe tile (single output DMA); host un-transposes during
    the unshard.  Measured cost: 0.833ns/weight-col (LDWEIGHTS, no
    FWL in bass) + ~30ns/matmul instruction.
  - DVE path (v1 fp16 chunked multiply + halving-tree reduce,
    ~162ns/position-row quiet): 3 partition tiles; tile 0 is the HEAD
    [0, XHEAD) of slots 128..255 whose tails [XHEAD, E) run on the PE
    as single-matmul strips, tiles 1-2 are slots 256..511 in full.
    XHEAD is picked at prep time to equalize predicted engine busy
    (~36us each) just under the ~38us DMA roofline (14.9MB/core fp16
    keys at the ~420GB/s measured HBM rate).  qp broadcasts straight
    from a [P,1,KDA] tile (stride-0 over s) -- no per-chunk ACT copy.
  - DMA: ALL inputs (headers + both key streams) issue on the SWDGE
    (gpsimd) queue -- fire-and-forget descriptor generation, drained
    in issue order at full rate -- explicitly interleaved by compute
    need-time (kt 2us early, first small pk chunk 2.6us early) so
    both engines stream without stalls.  Every issue is dependency-
    free: the DVE key tiles are fully SBUF-resident (3 tiles,
    ~58KB/partition) and chunks land in disjoint subtile slices.
    HWDGE dma_starts BLOCK their issuing engine's FIFO until the
    transfer completes (v2/v3/v5 lessons), so the HWDGE rings carry
    only outputs: per-tile e outs on sync, the combined PE out on
    scalar, each ~2us ring receipt, off the critical path.  Both key
    streams taper their final chunks to shorten the post-DMA tail.
  - Softmax normalization (and masking) on the host during unshard.

  Measured (8 cores, max over cores): 63.9-65.5us in quiet epochs,
  64-71us under the shared-box clock throttle (v1 baseline: 93.4us
  quiet / 111.5us throttled; DVE-bound).  Rel err 1.0e-2 (gate 2e-2).
"""

import sys

import numpy as np

sys.path.insert(0, "/opt/trn_rl_repo")

import concourse.bass as bass
import concourse.tile as tile
from concourse import bacc, mybir
from concourse.bass_utils import run_bass_kernel_spmd


def _install_trace_shims():
    """The agent image lacks ``antenv.axon_hooks``, so trace=True silently
    degrades.  Recreate the module and register the ctypes NTFF hook from
    trn_agent_boot; also make artifact upload failure non-fatal."""
    try:
        import types

        import antenv
        from concourse import bass_utils as _bu

        if "antenv.axon_hooks" not in sys.modules:
            mod = types.ModuleType("antenv.axon_hooks")
            mod._hook = None
            mod.set_axon_ntff_profile_hook = lambda h: setattr(mod, "_hook", h)
            mod.get_axon_ntff_profile_hook = lambda: mod._hook
            sys.modules["antenv.axon_hooks"] = mod
            antenv.axon_hooks = mod
            from trn_agent_boot.trn_boot import _ntff_profile_via_ctypes

            mod.set_axon_ntff_profile_hook(
                _ntff_profile_via_ctypes("/opt/axon/libaxon_pjrt.so")
            )

        _orig_upload = _bu.upload_artifacts

        def _safe_upload(tmpdir):
            try:
                return _orig_upload(tmpdir)
            except Exception:
                return "local://" + str(tmpdir)

        _bu.upload_artifacts = _safe_upload
    except Exception:
        pass


_install_trace_shims()

B, S, KD, QD = 4096, 200, 128, 128
NCORES = 8
P = 128
PB = B // NCORES           # batches per core (512)
CH = 38                    # s-positions per DVE keys DMA chunk (9.9KB/part descriptors)
KDA = KD + 2               # zero-padded to 130 (v1: bank-stagger stride)

PE_NSLOTS = 256            # per core; slots 128.. are strip-split at XHEAD
PE_GROUP = 128
PE_NGROUPS = PE_NSLOTS // PE_GROUP
PE_CHUNK_MAX = 4992        # cols per keysT DMA chunk (9.75KB/partition)
PE_CHUNK_FIRST = 1664      # small first chunk so the PE starts early
NTILES_DVE = 3             # head tile (slots 128..255) + slots 256..511

# measured cost constants (v2/v3 traces) for scheduling + balance
_PE_NS_PER_COL = 0.833
_PE_NS_PER_MM = 30.0
_DVE_NS_PER_POS = 163.0

LAST_RESULTS = None
_nc_cache = {}


def _dve_chunks(j, E, last):
    """Chunk schedule for DVE tile j: geometric ramp-up on tile 0 so the
    DVE starts as soon as the first keys land; ramp-DOWN at the end of
    the last tile so the post-DMA compute tail is short."""
    out = []
    c0 = 0
    if j == 0:
        for ch in (8, 16, 26):
            if c0 + ch > E:
                break
            out.append((c0, ch))
            c0 += ch
    tail = []
    rem_end = E
    if last:
        for ch in (8, 16, 26):
            if rem_end - ch <= c0:
                break
            tail.append((rem_end - ch, ch))
            rem_end -= ch
        tail.reverse()
    while c0 < rem_end:
        ch = min(CH, rem_end - c0)
        out.append((c0, ch))
        c0 += ch
    return out + tail


def _pe_widths(pe_exts, xhead):
    """Per-slot weight-column counts: full extent for slots < 128, the
    [xhead, E) strip for slots >= 128."""
    w = []
    for i, e in enumerate(pe_exts):
        w.append(e if i < PE_GROUP else max(0, e - xhead))
    return w


def _pe_chunks(widths):
    """Pack PE slots into DMA chunks of <= PE_CHUNK_MAX cols (slot-
    aligned), tapering the last chunks.  Returns (chunks, off)."""
    n = len(widths)
    total = sum(widths)
    targets = [PE_CHUNK_FIRST, PE_CHUNK_FIRST + 832]
    rem = total - sum(targets)
    while rem > PE_CHUNK_MAX * 1.5:
        targets.append(PE_CHUNK_MAX)
        rem -= PE_CHUNK_MAX
    targets += [int(rem * 0.5), int(rem * 0.3), rem]
    chunks = []
    off = [0] * n
    lo = 0
    cols = 0
    ti = 0
    for i, e in enumerate(widths):
        if cols + e > targets[min(ti, len(targets) - 1)] and cols > 0:
            chunks.append((lo, i, cols))
            ti += 1
            lo, cols = i, 0
        off[i] = cols
        cols += e
    chunks.append((lo, n, cols))
    return chunks, off


def _dma_order(pe_chunks, widths, dve_sched, skip=()):
    """Merge the two key streams by compute need-time (cumulative engine
    busy before each chunk is consumed).  Returns a list of
    ('pk', n) / ('kt', j, ci) in SWDGE issue order.  Units in `skip`
    are carried on the HWDGE rings instead (their clock still
    advances)."""
    units = []
    # kt chunks get a 2us head start so the DVE (which starts later and
    # has no deep chunk buffer ahead of it) never waits at a boundary;
    # the small first pk chunk goes out even earlier so the PE's first
    # matmuls aren't starved behind the kt ramp.
    t = -2000.0
    for j in range(NTILES_DVE):
        for ci, (c0, ch) in enumerate(dve_sched[j]):
            if ("kt", j, ci) not in skip:
                units.append((t, 0, ("kt", j, ci)))
            t += ch * _DVE_NS_PER_POS
    t = -2600.0
    for n, (lo, hi, cols) in enumerate(pe_chunks):
        if ("pk", n) not in skip:
            units.append((t, 1, ("pk", n)))
        nmm = sum(
            (2 if (i < PE_GROUP and widths[i] > P) else 1)
            for i in range(lo, hi)
            if widths[i] > 0
        )
        t += cols * _PE_NS_PER_COL + nmm * _PE_NS_PER_MM
    units.sort(key=lambda u: (u[0], u[1]))
    return [u[2] for u in units]


def _build(dve_exts, pe_exts, xhead):
    f16 = mybir.dt.float16
    f32 = mybir.dt.float32
    mult = mybir.AluOpType.mult
    add = mybir.AluOpType.add
    nc = bacc.Bacc("TRN2", target_bir_lowering=False, debug=False)

    widths = _pe_widths(pe_exts, xhead)
    pe_chunks, pe_off = _pe_chunks(widths)
    brow0 = max(0, max(pe_exts[:PE_GROUP]) - P)

    # ---- DRAM tensors
    pk_d = [
        nc.dram_tensor(f"pk{n}", [P, cols], f16, kind="ExternalInput")
        for n, (_, _, cols) in enumerate(pe_chunks)
    ]
    # combined header: [ w^T | queryT(PE) | qw(DVE, 3 tiles) ]
    HW = KD + PE_NSLOTS
    hdr_d = nc.dram_tensor(
        "hdr", [QD, HW + NTILES_DVE * (P + KD)], f16, kind="ExternalInput"
    )
    # single combined PE output: [eA0 | eA1 | eB0] column blocks
    pe_ncols = PE_NGROUPS * PE_GROUP + (PE_GROUP if brow0 > 0 else 0)
    pe_e_d = nc.dram_tensor("pe_e", [P, pe_ncols], f32, kind="ExternalOutput")
    keys_c = {}
    dve_sched = {}
    for j in range(NTILES_DVE):
        dve_sched[j] = _dve_chunks(j, dve_exts[j], j == NTILES_DVE - 1)
        for ci, (c0, ch) in enumerate(dve_sched[j]):
            keys_c[(j, ci)] = nc.dram_tensor(
                f"k{j}_{ci}", [P, ch, KDA], f16, kind="ExternalInput"
            )
    e_d = nc.dram_tensor("e", [NTILES_DVE * P, S], f32, kind="ExternalOutput")

    with tile.TileContext(nc) as tc:
        with (
            tc.tile_pool(name="pek", bufs=1) as pek,
            tc.tile_pool(name="keys", bufs=1) as keysp,
            tc.tile_pool(name="prod", bufs=2) as prodp,
            tc.tile_pool(name="tree", bufs=2) as treep,
            tc.tile_pool(name="small", bufs=2) as smallp,
            tc.tile_pool(name="qpp", bufs=NTILES_DVE) as qpp,
            tc.tile_pool(name="pemisc", bufs=1) as pemisc,
            tc.tile_pool(name="psum", bufs=2, space=bass.MemorySpace.PSUM) as psump,
            tc.tile_pool(name="pepsum", bufs=1, space=bass.MemorySpace.PSUM) as pepsum,
        ):
            # ---- header loads.  wq rides the scalar HWDGE ring ALONE:
            # a HWDGE dma_start blocks its issuing engine's FIFO until
            # the transfer completes, and the ACT's next instruction
            # (the qpT copy) waits on wq anyway.  qw rides sync, ahead
            # of the output DMAs.
            ktiles = [
                pek.tile([P, cols], f16, tag=f"pk{n}", name=f"pkt{n}")
                for n, (_, _, cols) in enumerate(pe_chunks)
            ]
            dtiles = [
                keysp.tile(
                    [P, dve_exts[j], KDA], f16, tag=f"kt{j}", name=f"dkt{j}"
                )
                for j in range(NTILES_DVE)
            ]
            # ---- EVERYTHING (headers + all key chunks) on SWDGE in
            # need-time order -- HWDGE dma_starts block their issuing
            # engine and delay the SWDGE stream start; with no HWDGE
            # input traffic the SDMA engines reach the SWDGE queue as
            # soon as the Q7 emits descriptors.  Every issue is
            # dependency-free (distinct PE chunk tiles; fully resident
            # DVE key tiles with chunks in disjoint subtile slices).
            hdr = pemisc.tile(
                [QD, HW + NTILES_DVE * (P + KD)], f16, tag="hdr", name="hdr"
            )
            h1 = HW + (P + KD)
            nc.gpsimd.dma_start(hdr[:, 0:h1], hdr_d[:, 0:h1])
            wq = hdr[:, 0:HW]
            hdr1_emitted = False
            for idx, unit in enumerate(
                _dma_order(pe_chunks, widths, dve_sched)
            ):
                if idx == 3 and not hdr1_emitted:
                    nc.gpsimd.dma_start(hdr[:, h1:], hdr_d[:, h1:])
                    hdr1_emitted = True
                if unit[0] == "pk":
                    n = unit[1]
                    nc.gpsimd.dma_start(ktiles[n][:], pk_d[n][:])
                else:
                    _, j, ci = unit
                    c0, ch = dve_sched[j][ci]
                    nc.gpsimd.dma_start(
                        dtiles[j][:, c0 : c0 + ch, :], keys_c[(j, ci)][:]
                    )
            if not hdr1_emitted:
                nc.gpsimd.dma_start(hdr[:, h1:], hdr_d[:, h1:])

            # ---- qpT for the PE path: qpT[k, i] = sum_q w[k,q] qT[q, i]
            qpT_ps = pepsum.tile([P, PE_NSLOTS], f32, tag="qpT_ps")
            nc.tensor.matmul(
                qpT_ps[:], wq[:, :KD], wq[:, KD:], start=True, stop=True
            )
            qpT = pemisc.tile([P, PE_NSLOTS], f16, tag="qpT")
            nc.scalar.copy(qpT[:], qpT_ps[:])  # f32 -> f16 on ACT

            # ---- DVE-path qp per tile (3D so it can broadcast over s)
            qps = []
            for j in range(NTILES_DVE):
                qb = HW + j * (P + KD)
                qp_ps = psump.tile([P, KD], f32, tag="qp_ps")
                nc.tensor.matmul(
                    qp_ps[:], hdr[:, qb : qb + P], hdr[:, qb + P : qb + P + KD],
                    start=True, stop=True,
                )
                qp = qpp.tile([P, 1, KDA], f16, tag=f"qp{j}", name=f"qp{j}")
                nc.scalar.copy(qp[:, 0, :KD], qp_ps[:])
                qps.append(qp)

            # ---- PE per-slot matmuls (PE queue only)
            psA = [
                pepsum.tile([P, PE_GROUP], f32, tag=f"psA{g}", name=f"psA{g}")
                for g in range(PE_NGROUPS)
            ]
            psB0 = (
                pepsum.tile([brow0, PE_GROUP], f32, tag="psB0", name="psB0")
                if brow0 > 0
                else None
            )
            for n, (lo, hi, cols) in enumerate(pe_chunks):
                kt = ktiles[n]
                for i in range(lo, hi):
                    wd = widths[i]
                    if wd <= 0:
                        continue
                    g = i // PE_GROUP
                    col = i % PE_GROUP
                    o = pe_off[i]
                    if g == 0:
                        ea = min(wd, P)
                        nc.tensor.matmul(
                            psA[0][0:ea, col : col + 1],
                            kt[:, o : o + ea],
                            qpT[:, i : i + 1],
                            start=True, stop=True,
                        )
                        if wd > P:
                            nc.tensor.matmul(
                                psB0[0 : wd - P, col : col + 1],
                                kt[:, o + P : o + wd],
                                qpT[:, i : i + 1],
                                start=True, stop=True,
                            )
                    else:
                        nc.tensor.matmul(
                            psA[1][0:wd, col : col + 1],
                            kt[:, o : o + wd],
                            qpT[:, i : i + 1],
                            start=True, stop=True,
                        )

            # ---- DVE-path main loop
            for j in range(NTILES_DVE):
                E = dve_exts[j]
                qp = qps[j]
                kt = dtiles[j]
                att = smallp.tile([P, E], f32, tag="att", bufs=3)
                e_t = smallp.tile([P, E], f32, tag="e", bufs=3)
                for ci, (c0, ch) in enumerate(dve_sched[j]):
                    prod = prodp.tile([P, CH, KDA], f16, tag="prod")
                    qp_b = qp[:, 0:1, 0:KD].broadcast_to([P, ch, KD])
                    nc.vector.tensor_tensor(
                        prod[:, :ch, 0:KD], kt[:, c0 : c0 + ch, 0:KD], qp_b,
                        op=mult,
                    )
                    r1 = treep.tile([P, CH, 64], f16, tag="r1")
                    nc.vector.tensor_tensor(
                        r1[:, :ch, :], prod[:, :ch, 0:64], prod[:, :ch, 64:128],
                        op=add,
                    )
                    r2 = treep.tile([P, CH, 32], f16, tag="r2")
                    nc.vector.tensor_tensor(
                        r2[:, :ch, :], r1[:, :ch, 0:32], r1[:, :ch, 32:64],
                        op=add,
                    )
                    r3 = treep.tile([P, CH, 16], f16, tag="r3")
                    nc.vector.tensor_tensor(
                        r3[:, :ch, :], r2[:, :ch, 0:16], r2[:, :ch, 16:32],
                        op=add,
                    )
                    r4 = treep.tile([P, CH, 8], f16, tag="r4")
                    nc.vector.tensor_tensor(
                        r4[:, :ch, :], r3[:, :ch, 0:8], r3[:, :ch, 8:16],
                        op=add,
                    )
                    nc.vector.tensor_reduce(
                        att[:, c0 : c0 + ch], r4[:, :ch, :],
                        axis=mybir.AxisListType.X, op=add,
                    )
                    nc.scalar.activation(
                        e_t[:, c0 : c0 + ch],
                        att[:, c0 : c0 + ch],
                        mybir.ActivationFunctionType.Exp,
                        bias=0.0,
                        scale=1.0,
                    )
                # one output DMA per tile on the sync HWDGE ring (the
                # SWDGE Q7 must stay free to generate input descriptors
                # back-to-back -- in v4 the out-issues' exp waits stalled
                # the input stream)
                nc.sync.dma_start(e_d[j * P : (j + 1) * P, 0:E], e_t[:, 0:E])

            # ---- PE-path exp into one combined tile + single out
            e_pe = pemisc.tile([P, pe_ncols], f32, tag="e_pe")
            for g in range(PE_NGROUPS):
                nc.scalar.activation(
                    e_pe[:, g * PE_GROUP : (g + 1) * PE_GROUP], psA[g][:],
                    mybir.ActivationFunctionType.Exp, bias=0.0, scale=1.0,
                )
            if psB0 is not None:
                nc.scalar.activation(
                    e_pe[0:brow0, 2 * PE_GROUP : 3 * PE_GROUP],
                    psB0[0:brow0, :],
                    mybir.ActivationFunctionType.Exp, bias=0.0, scale=1.0,
                )
            # scalar ring: runs in parallel with the last e-outs on sync
            nc.scalar.dma_start(pe_e_d[:], e_pe[:])
    nc.compile()
    return nc


def _balance_xhead(pe_exts_full, e2, e3):
    """Pick XHEAD (even) minimizing max(predicted PE busy, DVE busy)."""
    best = (float("inf"), 64)
    for x in range(32, 128, 2):
        cols = sum(pe_exts_full[:PE_GROUP]) + sum(
            max(0, e - x) for e in pe_exts_full[PE_GROUP:]
        )
        mms = (
            PE_GROUP
            + sum(1 for e in pe_exts_full[:PE_GROUP] if e > P)
            + sum(1 for e in pe_exts_full[PE_GROUP:] if e > x)
        )
        pe = _PE_NS_PER_COL * cols + _PE_NS_PER_MM * mms
        dve = _DVE_NS_PER_POS * (x + e2 + e3)
        m = max(pe, dve)
        if m < best[0]:
            best = (m, x)
    return best[1]


def _prep(query, keys, seq_len, w):
    query = np.ascontiguousarray(np.asarray(query), dtype=np.float32)
    keys = np.asarray(keys)
    w = np.ascontiguousarray(np.asarray(w), dtype=np.float32)
    lens = np.asarray(seq_len).reshape(B).astype(np.int64)

    order = np.argsort(-lens, kind="stable")
    keys16 = keys.astype(np.float16)
    wT16 = np.ascontiguousarray(w.T.astype(np.float16))  # [q, k]
    query16 = query.astype(np.float16)

    def slot_ext(s):
        l = int(lens[order[NCORES * s]])
        return max(2, l + (l & 1))

    pe_exts = tuple(slot_ext(i) for i in range(PE_NSLOTS))
    e2 = min(S, max(1, slot_ext(256)))
    e3 = min(S, max(1, slot_ext(384)))
    xhead = _balance_xhead(pe_exts, e2, e3)
    dve_exts = (xhead, e2, e3)
    widths = _pe_widths(pe_exts, xhead)
    pe_chunks, pe_off = _pe_chunks(widths)
    dve_sched = {
        j: _dve_chunks(j, dve_exts[j], j == NTILES_DVE - 1)
        for j in range(NTILES_DVE)
    }

    in_maps = []
    pe_batches = []
    dve_batches = []
    for c in range(NCORES):
        slots = order[c::NCORES]  # slot s -> batch order[8s + c]
        pb = slots[:PE_NSLOTS]
        db = slots[P:].copy()     # DVE tiles: slots 128..511
        pe_batches.append(pb)
        dve_batches.append(db)

        im = {}
        # combined header: [ w^T | queryT(PE) | qw(DVE) flattened ]
        HW = KD + PE_NSLOTS
        hdr = np.zeros((QD, HW + NTILES_DVE * (P + KD)), dtype=np.float16)
        hdr[:, :KD] = wT16
        hdr[:, KD:HW] = query16[pb, 0, :].T
        qTd = query[db, 0, :].reshape(NTILES_DVE, P, QD).transpose(2, 0, 1)
        for j in range(NTILES_DVE):
            qb = HW + j * (P + KD)
            hdr[:, qb : qb + P] = qTd[:, j, :]
            hdr[:, qb + P : qb + P + KD] = wT16
        im["hdr"] = hdr
        for n, (lo, hi, cols) in enumerate(pe_chunks):
            blk = np.zeros((P, cols), dtype=np.float16)
            for i in range(lo, hi):
                wd = widths[i]
                if wd <= 0:
                    continue
                b = pb[i]
                l = int(lens[b])
                s0 = 0 if i < PE_GROUP else xhead
                if l > s0:
                    o = pe_off[i]
                    blk[:, o : o + (l - s0)] = keys16[b, s0:l, :].T
            im[f"pk{n}"] = blk
        # DVE side
        keys_aug = np.zeros((NTILES_DVE * P, S, KDA), dtype=np.float16)
        keys_aug[:, :, :KD] = keys16[db]
        for j in range(NTILES_DVE):
            for ci, (c0, ch) in enumerate(dve_sched[j]):
                im[f"k{j}_{ci}"] = np.ascontiguousarray(
                    keys_aug[j * P : (j + 1) * P, c0 : c0 + ch, :]
                )
        in_maps.append(im)
    return lens, dve_exts, pe_exts, xhead, pe_batches, dve_batches, in_maps


def kernel(query, keys, seq_len, w):
    global LAST_RESULTS
    (lens, dve_exts, pe_exts, xhead, pe_batches, dve_batches, in_maps) = _prep(
        query, keys, seq_len, w
    )

    key = (dve_exts, pe_exts, xhead)
    nc = _nc_cache.get(key)
    if nc is None:
        nc = _build(dve_exts, pe_exts, xhead)
        _nc_cache[key] = nc

    res = run_bass_kernel_spmd(nc, in_maps, core_ids=list(range(NCORES)))
    LAST_RESULTS = res

    out = np.zeros((B, S), dtype=np.float32)
    for c in range(NCORES):
        r = res.results[c]
        pb = pe_batches[c]
        db = dve_batches[c]
        pe_e = np.asarray(r["pe_e"])
        peA = [
            pe_e[:, g * PE_GROUP : (g + 1) * PE_GROUP]
            for g in range(PE_NGROUPS)
        ]
        peB0 = (
            pe_e[:, 2 * PE_GROUP : 3 * PE_GROUP]
            if pe_e.shape[1] > 2 * PE_GROUP
            else None
        )
        e = np.asarray(r["e"])
        # PE group 0: full rows
        for i in range(PE_GROUP):
            b = pb[i]
            l = int(lens[b])
            if l == 0:
                continue
            if l <= P:
                v = peA[0][:l, i]
            else:
                v = np.concatenate([peA[0][:, i], peB0[: l - P, i]])
            ssum = float(v.sum())
            if ssum == 0.0 or not np.isfinite(ssum):
                ssum = 1.0
            out[b, :l] = v / ssum
        # PE group 1 strips + DVE head tile (tile 0)
        for i in range(PE_GROUP, PE_NSLOTS):
            b = pb[i]
            l = int(lens[b])
            if l == 0:
                continue
            col = i - PE_GROUP
            lh = min(l, xhead)
            head = e[col, :lh]
            if l > xhead:
                v = np.concatenate([head, peA[1][: l - xhead, col]])
            else:
                v = head
            ssum = float(v.sum())
            if ssum == 0.0 or not np.isfinite(ssum):
                ssum = 1.0
            out[b, :l] = v / ssum
        # DVE tiles 1..2 (slots 256..511): plain softmax rows
        for j in range(1, NTILES_DVE):
            E = dve_exts[j]
            rows = db[j * P : (j + 1) * P]
            blk = e[j * P : (j + 1) * P, :E]
            m = (np.arange(E)[None, :] < lens[rows][:, None]).astype(np.float32)
            blk = np.where(m > 0, blk, 0.0)
            ssum = blk.sum(axis=1, keepdims=True)
            ssum[ssum == 0.0] = 1.0
            out[rows, :E] = blk / ssum
    out[lens == 0, :] = np.float32(1.0 / S)
    return out
